# revision 1
# baseline (speedup 1.0000x reference)
"""Trainium2 Bass kernel for modded-nanogpt CausalSelfAttention, 8-way
tensor-parallel over heads.

Per core (4 of 32 heads, local width F=128):
  - QKV projection in [t, f] layout: lhsT = x^T d-tiles (stationary),
    rhs = [wq|wk|wv]^T slice -> psum [t128, 384].
  - RMS-norm + half-truncated rotary on q,k via DVE; rsqrt = exp(-0.5*ln(x))
    so the whole kernel uses one ACT table set (natural_log_exp).
  - q,k transposed to [f, t] with DMA-transpose; v merged with lambdas*ve and
    stored augmented with a ones column ([v_h | 1] per head) for softmax sums.
  - Scores S^T[j, i] per head with K=32 row-tiled matmuls (4 heads concurrent
    in the PE array via tile_position), exp on ACT (scale=0.12 folded in),
    causal handled block-granular + triangle mask on diagonal blocks.
  - attn@v: lhsT = vaug (M=33: 32 v cols + ones), rhs = P^T -> y^T and the
    softmax denominator in one accumulation; normalize with DVE reciprocal +
    DMA partition-broadcast.
  - c_proj on the local 128 e-columns -> partial [t, d] output; partials are
    summed across the 8 cores on the host.
"""
import os
import sys

if os.path.isdir("/opt/trn_rl_repo") and "/opt/trn_rl_repo" not in sys.path:
    sys.path.insert(0, "/opt/trn_rl_repo")

from contextlib import ExitStack

import ml_dtypes
import numpy as np

import concourse.bass as bass
import concourse.tile as tile
from concourse import bacc, mybir
from concourse.bass_utils import run_bass_kernel_spmd

F32 = mybir.dt.float32
BF16 = mybir.dt.bfloat16
AF = mybir.ActivationFunctionType
ALU = mybir.AluOpType

N_CORES = 8
T = 2048
D = 1024
HD = 32          # head dim
HC = 4           # heads per core
F = HC * HD      # local qkv width = 128
TT = T // 128    # 16 t-tiles
DT = D // 128    # 8 d-tiles
CW = 512         # i-chunk width
NCH = T // CW    # 4 i-chunks
SCALE = 0.12
RMS_EPS = 1.1920929e-07
ROT_BASE = HD * 4


def _ap(a, offset_delta, dims):
    """Re-strided view of tile AP `a`: keep partition dim, replace free dims."""
    return bass.AP(tensor=a.tensor, offset=a.offset + offset_delta,
                   ap=[list(a.ap[0])] + [list(d) for d in dims])


def _bcast_part(a, n):
    """View of AP `a` with the partition dim replaced by an n-way broadcast."""
    return bass.AP(tensor=a.tensor, offset=a.offset,
                   ap=[[0, n]] + [list(d) for d in a.ap[1:]])


def build_program(repeats: int = 1):
    nc = bacc.Bacc("TRN2", target_bir_lowering=False, debug=False,
                   num_devices=N_CORES)

    xt = nc.dram_tensor("xt", [D, T], BF16, kind="ExternalInput").ap()
    wqkv = nc.dram_tensor("wqkv", [D, 3 * F], BF16, kind="ExternalInput").ap()
    vein = nc.dram_tensor("vein", [T, F], F32, kind="ExternalInput").ap()
    cpw = nc.dram_tensor("cpw", [F, D], BF16, kind="ExternalInput").ap()
    lam = nc.dram_tensor("lam", [128, 2], F32, kind="ExternalInput").ap()
    rota = nc.dram_tensor("rota", [T, HD], BF16, kind="ExternalInput").ap()
    rotb = nc.dram_tensor("rotb", [T, HD], BF16, kind="ExternalInput").ap()
    trimask = nc.dram_tensor("trimask", [128, 128], BF16,
                             kind="ExternalInput").ap()
    out = nc.dram_tensor("out", [T, D], F32, kind="ExternalOutput").ap()

    with tile.TileContext(nc) as tc, ExitStack() as ctx:
        const = ctx.enter_context(tc.tile_pool(name="const", bufs=1))
        x_sb = const.tile([128, DT, T], BF16)
        w_sb = const.tile([128, DT, 3 * F], BF16)
        cpw_sb = const.tile([128, D], BF16)
        lam_sb = const.tile([128, 2], F32)
        rota_sb = const.tile([128, TT, HD], BF16)
        rotb_sb = const.tile([128, TT, HD], BF16)
        tri_sb = const.tile([128, 128], BF16)
        eps_sb = const.tile([128, 1], F32)
        qT_sb = const.tile([128, T], BF16)
        kT_sb = const.tile([128, T], BF16)
        vaug_sb = const.tile([128, TT, 33 * HC], BF16)
        yT_sb = const.tile([128, T], BF16)

        def body(_iv=None):
            # ---- constant loads ----
            nc.sync.dma_start(x_sb[:], xt.rearrange("(n p) t -> p n t", p=128))
            nc.sync.dma_start(w_sb[:], wqkv.rearrange("(n p) f -> p n f", p=128))
            nc.sync.dma_start(cpw_sb[:], cpw)
            nc.sync.dma_start(lam_sb[:], lam)
            nc.sync.dma_start(rota_sb[:], rota.rearrange("(n p) f -> p n f", p=128))
            nc.sync.dma_start(rotb_sb[:], rotb.rearrange("(n p) f -> p n f", p=128))
            nc.sync.dma_start(tri_sb[:], trimask)
            nc.vector.memset(eps_sb[:], RMS_EPS)
            for h in range(HC):
                nc.vector.memset(vaug_sb[:, :, 33 * h + 32:33 * h + 33], 1.0)

            qk_all = const.tile([128, TT, 2 * F], BF16)
            ms_all = const.tile([128, TT, 8], F32)
            rs_all = const.tile([128, TT, 8], F32)
            with tc.tile_pool(name="qkv_ps", bufs=2, space="PSUM") as qkv_ps, \
                 tc.tile_pool(name="work", bufs=3) as work:
                # pass 1: projections + squared-sum stats + v merge
                for tt in range(TT):
                    ps = qkv_ps.tile([128, 3 * F], F32)
                    for dt in range(DT):
                        nc.tensor.matmul(
                            ps[:], lhsT=x_sb[:, dt, 128 * tt:128 * (tt + 1)],
                            rhs=w_sb[:, dt, :],
                            start=(dt == 0), stop=(dt == DT - 1))
                    nc.vector.tensor_copy(qk_all[:, tt, :], ps[:, 0:2 * F])
                    sq = work.tile([128, 2 * F], F32, tag="sq")
                    nc.vector.tensor_mul(sq[:], qk_all[:, tt, :],
                                         qk_all[:, tt, :])
                    nc.vector.reduce_sum(
                        ms_all[:, tt, :],
                        sq[:].rearrange("p (g d) -> p g d", g=8),
                        axis=mybir.AxisListType.X)

                    # --- v: lam0*v + lam1*ve, augmented layout ---
                    vet = work.tile([128, F], F32, tag="vet")
                    nc.sync.dma_start(vet[:], vein[128 * tt:128 * (tt + 1), :])
                    vel = work.tile([128, F], F32, tag="vel")
                    nc.vector.tensor_scalar_mul(vel[:], vet[:], lam_sb[:, 1:2])
                    vaug_v = _ap(vaug_sb[:, tt, :], 0, [[33, HC], [1, HD]])
                    nc.vector.scalar_tensor_tensor(
                        out=vaug_v, in0=ps[:, 2 * F:3 * F],
                        scalar=lam_sb[:, 0:1], in1=vel[:],
                        op0=ALU.mult, op1=ALU.add)

                # pass 2: one batched ln + exp -> rstd for all tiles.
                # Single Ln before every Exp in the program => one ACT
                # table load (natural_log_exp set covers both).
                nc.scalar.activation(rs_all[:], ms_all[:], AF.Ln,
                                     bias=eps_sb[:], scale=1.0 / HD)
                nc.scalar.activation(rs_all[:], rs_all[:], AF.Exp, scale=-0.5)

                # pass 3: rotary + norm-apply + transpose to [f, t]
                for tt in range(TT):
                    qk = qk_all[:, tt, :]
                    ta = work.tile([128, 2 * F], BF16, tag="ta")
                    rota_v = _ap(rota_sb[:, tt, :], 0, [[0, 8], [1, HD]])
                    nc.vector.tensor_mul(ta[:], qk, rota_v)
                    tb = work.tile([128, 2 * F], BF16, tag="tb")
                    qk_swap = _ap(qk, 16, [[32, 8], [-16, 2], [1, 16]])
                    rotb_v = _ap(rotb_sb[:, tt, :], 0, [[0, 8], [1, HD]])
                    nc.vector.tensor_mul(tb[:], qk_swap, rotb_v)
                    rot = work.tile([128, 2 * F], BF16, tag="rot")
                    nc.vector.tensor_add(rot[:], ta[:], tb[:])
                    qkn = work.tile([128, 2 * F], BF16, tag="qkn")
                    rs_v = _ap(rs_all[:, tt, :], 0, [[1, 8], [0, HD]])
                    nc.vector.tensor_mul(qkn[:], rot[:], rs_v)

                    nc.sync.dma_start_transpose(
                        qT_sb[:, 128 * tt:128 * (tt + 1)], qkn[:, 0:F])
                    nc.sync.dma_start_transpose(
                        kT_sb[:, 128 * tt:128 * (tt + 1)], qkn[:, F:2 * F])

            # ---- attention ----
            with tc.tile_pool(name="sc_ps", bufs=4, space="PSUM") as sc_ps, \
                 tc.tile_pool(name="y_ps", bufs=1, space="PSUM") as y_ps_pool, \
                 tc.tile_pool(name="pt", bufs=4) as pt_pool, \
                 tc.tile_pool(name="nrm", bufs=4) as nrm:
                for c in range(NCH):
                    y_ps = [y_ps_pool.tile([33, CW], F32, name=f"y{h}",
                                           tag=f"y{h}")
                            for h in range(HC)]
                    njt = 4 * c + 4
                    for jt in range(njt):
                        q0 = max(0, jt - 4 * c)
                        nt = CW - 128 * q0
                        i_lo = CW * c + 128 * q0
                        for h in range(HC):
                            sc = sc_ps.tile([128, CW], F32, tag="sc")
                            pt = pt_pool.tile([128, CW], BF16, tag="pt")
                            nc.tensor.matmul(
                                sc[:, 0:nt],
                                lhsT=kT_sb[32 * h:32 * (h + 1),
                                           128 * jt:128 * (jt + 1)],
                                rhs=qT_sb[32 * h:32 * (h + 1), i_lo:CW * (c + 1)],
                                start=True, stop=True,
                                tile_position=(32 * h, 0))
                            nc.scalar.activation(pt[:, 0:nt], sc[:, 0:nt],
                                                 AF.Exp, scale=SCALE)
                            if jt >= 4 * c:
                                nc.gpsimd.tensor_mul(pt[:, 0:128],
                                                     pt[:, 0:128], tri_sb[:])
                            nc.tensor.matmul(
                                y_ps[h][:, 128 * q0:CW],
                                lhsT=vaug_sb[:, jt, 33 * h:33 * (h + 1)],
                                rhs=pt[:, 0:nt],
                                start=(jt == 0), stop=(jt == njt - 1))
                    for h in range(HC):
                        rl = nrm.tile([1, CW], F32, tag="rl")
                        nc.vector.reciprocal(rl[:], y_ps[h][32:33, :])
                        rlb = nrm.tile([32, CW], F32, tag="rlb")
                        nc.gpsimd.partition_broadcast(rlb[:], rl[:])
                        nc.vector.tensor_mul(
                            yT_sb[32 * h:32 * (h + 1), CW * c:CW * (c + 1)],
                            y_ps[h][0:32, :], rlb[:])

            # ---- c_proj ----
            with tc.tile_pool(name="o_ps", bufs=4, space="PSUM") as o_ps, \
                 tc.tile_pool(name="o_sb", bufs=4) as o_sb:
                for ti in range(TT):
                    for dc in range(2):
                        op = o_ps.tile([128, CW], F32)
                        nc.tensor.matmul(
                            op[:], lhsT=yT_sb[:, 128 * ti:128 * (ti + 1)],
                            rhs=cpw_sb[:, CW * dc:CW * (dc + 1)],
                            start=True, stop=True)
                        ob = o_sb.tile([128, CW], F32, tag="ob")
                        nc.vector.tensor_copy(ob[:], op[:])
                        nc.sync.dma_start(
                            out[128 * ti:128 * (ti + 1), CW * dc:CW * (dc + 1)],
                            ob[:])

        if repeats == 1:
            body()
        else:
            with tc.For_i(0, repeats, 1) as _i:
                body(_i)

    nc.compile()
    return nc


def _host_inputs(x, ve, qkv_w, lambdas, c_proj_w):
    bf = ml_dtypes.bfloat16
    x2 = np.asarray(x, np.float32).reshape(T, D)
    xt = np.ascontiguousarray(x2.T).astype(bf)
    ve2 = np.asarray(ve, np.float32).reshape(T, D)
    qkv = np.asarray(qkv_w, np.float32)
    cp = np.asarray(c_proj_w, np.float32)
    lamv = np.asarray(lambdas, np.float32).reshape(2)
    lam_b = np.broadcast_to(lamv, (128, 2)).copy()

    quarter = HD // 4
    af = (np.float32(1.0) / np.float32(ROT_BASE)) ** np.linspace(
        0.0, 1.0, quarter, dtype=np.float32)
    af = np.concatenate([af, np.zeros(quarter, np.float32)])
    theta = np.arange(T, dtype=np.float32)[:, None] * af[None, :]
    cos = np.cos(theta).astype(np.float32)
    sin = np.sin(theta).astype(np.float32)
    rota = np.concatenate([cos, cos], axis=1).astype(bf)   # [T, 32]
    rotb = np.concatenate([sin, -sin], axis=1).astype(bf)  # [T, 32]

    jj = np.arange(128)[:, None]
    ii = np.arange(128)[None, :]
    trim = (ii >= jj).astype(bf)                       # keep j <= i

    in_maps = []
    for cidx in range(N_CORES):
        sl = slice(F * cidx, F * (cidx + 1))
        wq = qkv[0][sl, :]
        wk = qkv[1][sl, :]
        wv = qkv[2][sl, :]
        wcat = np.ascontiguousarray(
            np.concatenate([wq, wk, wv], axis=0).T).astype(bf)  # [1024, 384]
        in_maps.append({
            "xt": xt,
            "wqkv": wcat,
            "vein": np.ascontiguousarray(ve2[:, sl]),
            "cpw": np.ascontiguousarray(cp[:, sl].T).astype(bf),
            "lam": lam_b,
            "rota": rota,
            "rotb": rotb,
            "trimask": trim,
        })
    return in_maps


_NC_CACHE = {}


def _get_program(repeats: int = 1):
    if repeats not in _NC_CACHE:
        _NC_CACHE[repeats] = build_program(repeats)
    return _NC_CACHE[repeats]


def run_prepared(in_maps, repeats: int = 1):
    nc = _get_program(repeats)
    res = run_bass_kernel_spmd(nc, in_maps, list(range(N_CORES)))
    acc = np.zeros((T, D), np.float64)
    for cidx in range(N_CORES):
        acc += res.results[cidx]["out"].astype(np.float64)
    return acc.astype(np.float32).reshape(1, T, D)


def run(inputs, repeats: int = 1):
    nc = _get_program(repeats)
    in_maps = _host_inputs(**inputs)
    res = run_bass_kernel_spmd(nc, in_maps, list(range(N_CORES)))
    acc = np.zeros((T, D), np.float64)
    for cidx in range(N_CORES):
        acc += res.results[cidx]["out"].astype(np.float64)
    return acc.astype(np.float32).reshape(1, T, D)


def kernel(x, ve, qkv_w, lambdas, c_proj_w):
    return run(dict(x=x, ve=ve, qkv_w=qkv_w, lambdas=lambdas,
                    c_proj_w=c_proj_w))

